# revision 47
# baseline (speedup 1.0000x reference)
"""Trainium2 Bass kernel for the SelfAttentionBlock problem (8 NeuronCores).

Sharding strategy (unchanged from baseline):
  * MLP (q/k/v two-layer GELU blocks): data-parallel over rows — each core
    owns 256 tokens of each batch (512 rows total) and computes full-width
    q/k/v for those rows. Weights replicated per core.
  * Attention: head-parallel — core c computes head c for both batches.
    Rows->heads redistribution is a single AllToAll per tensor (q, k, v).
  * k/v/attention outputs are gathered and reassembled on the host.

Scheduling/structure (this version):
  * All weight DMAs are issued up front in need-order, with xT/w1 split
    into per-k-tile chunks, so the first matmul starts ~2us in (the old
    version idled the PE for 28us).
  * The attention score matmuls for q-chunks 0/1 are interleaved into the
    middle of the v-block, and chunks 2/3 right after it, so the PE stream
    never pauses across the a2a(v) boundary (keeps the HAM clock-gate at
    full rate; the old version ran the whole attention phase at half clock).
  * Scores for the two batches run CONCURRENTLY in the PE array via row
    tiling: batch 0's kT/qT live on SBUF partitions 0-63, batch 1's on
    64-127, and the two K=64 matmuls issue at tile_position (0,0)/(64,0),
    writing the two banks of one [128,1024] PSUM tile.  One exp activation
    then covers both batches (halves ACT instruction overhead).
  * Scores/exp/PV are trimmed to the causal triangle: a diagonal tile at
    offset o only computes columns [o:512].
  * Softmax normalization is moved to the host: the kernel ships the
    unnormalized PV output with the denominator row ([N, 65, S] per core)
    and the host divides.  This removes the very slow DVE reciprocals
    (3.3us each), the broadcast matmuls and casts of the old tail.
"""

import ml_dtypes
import numpy as np

import concourse.bass as bass
import concourse.mybir as mybir
from concourse import bacc, tile
from concourse.bass_utils import run_bass_kernel_spmd

N_CORES = 8
N, S, D, H = 2, 2048, 512, 2048
HEADS = 8
Dh = D // HEADS            # 64
RPC = S // N_CORES         # 256 rows per core per batch
ROWS = N * RPC             # 512 rows per core
KT1 = D // 128             # 4 layer-1 contraction tiles
MT1 = H // 128             # 16 layer-1 out tiles == layer-2 contraction tiles
MT2 = D // 128             # 4 layer-2 out tiles
QC = S // 512              # 4 q-chunks per batch

F32 = mybir.dt.float32
F32R = mybir.dt.float32r
BF16 = mybir.dt.bfloat16
AF = mybir.ActivationFunctionType


def _build():
    nc = bacc.Bacc("TRN2", target_bir_lowering=False, debug=False,
                   num_devices=N_CORES)

    xT = nc.dram_tensor("xT", [D, ROWS], BF16, kind="ExternalInput")
    w1 = {t: nc.dram_tensor(f"w1{t}", [D, H], BF16, kind="ExternalInput")
          for t in "kqv"}
    w2 = {t: nc.dram_tensor(f"w2{t}", [H, D], BF16, kind="ExternalInput")
          for t in "kqv"}
    b1 = {t: nc.dram_tensor(f"b1{t}", [H], F32R, kind="ExternalInput")
          for t in "kqv"}
    # b2 for k/q feeds DVE tensor_scalar_add, which requires plain float32
    b2 = {t: nc.dram_tensor(f"b2{t}", [D], F32 if t in "kq" else BF16,
                            kind="ExternalInput") for t in "kqv"}
    tri_d = nc.dram_tensor("tri", [128, 128], BF16, kind="ExternalInput")
    ones_col_d = nc.dram_tensor("ones_col", [128, MT1], BF16,
                                kind="ExternalInput")
    ones_row_d = nc.dram_tensor("ones_row", [1, 128], BF16,
                                kind="ExternalInput")
    kT_out = nc.dram_tensor("kT_out", [D, ROWS], BF16, kind="ExternalOutput")
    v_out = nc.dram_tensor("v_out", [ROWS, D], BF16, kind="ExternalOutput")
    po_out = nc.dram_tensor("attn_po", [N, Dh + 1, S], F32R,
                            kind="ExternalOutput")

    with tile.TileContext(nc) as tc:
        with (
            tc.tile_pool(name="dram", bufs=1, space="DRAM") as dp,
            tc.tile_pool(name="cst", bufs=1) as cst,
            tc.tile_pool(name="w1p", bufs=2) as w1p,
            tc.tile_pool(name="w2p", bufs=4) as w2p,
            tc.tile_pool(name="h1p", bufs=16) as h1p,
            tc.tile_pool(name="l2p", bufs=2) as l2p,
            tc.tile_pool(name="att", bufs=1) as attp,
            tc.tile_pool(name="vg", bufs=2) as vgp,
            tc.tile_pool(name="exp", bufs=16) as expp,
            tc.tile_pool(name="ex2", bufs=12) as expp2,
            tc.tile_pool(name="sm", bufs=2) as smp,
            tc.tile_pool(name="ps", bufs=2, space="PSUM") as psp,
            tc.tile_pool(name="pr", bufs=1, space="PSUM") as prp,
            tc.tile_pool(name="po", bufs=2, space="PSUM") as pop,
        ):
            send = {
                "k": dp.tile([D, ROWS], BF16, tag="send_k", name="send_k"),
                "q": dp.tile([D, ROWS], BF16, tag="send_q", name="send_q"),
                "v": dp.tile([N_CORES, ROWS, Dh], BF16, tag="send_v",
                             name="send_v"),
            }
            recv = {
                "k": dp.tile([D, ROWS], BF16, tag="recv_k", name="recv_k"),
                "q": dp.tile([D, ROWS], BF16, tag="recv_q", name="recv_q"),
                "v": dp.tile([N_CORES, ROWS, Dh], BF16, tag="recv_v",
                             name="recv_v"),
            }

            # ---- sync ring DMAs. The ring is one serial resource
            # (~bytes/358GB + ~1us per DMA) and a trigger whose dependency
            # is unmet blocks every later DMA on the ring, so order is
            # need-order: x + k weights + k biases first (first matmul at
            # ~18us incl the ~10.5us NEFF init), q next, v's weights last
            # (their pool-slot-reuse waits are harmless by then).
            w1_sb, w2_sb, b1_sb, b2qk_sb = {}, {}, {}, {}

            def load_w1(t, eng=None):
                eng = eng or nc.sync
                w1t = w1p.tile([128, KT1 * H], BF16, tag="w1", name=f"w1_{t}")
                for c in range(2):
                    eng.dma_start(
                        w1t[:, c * 2 * H:(c + 1) * 2 * H]
                        .rearrange("p (k h) -> p k h", k=2),
                        w1[t][c * 256:(c + 1) * 256, :]
                        .rearrange("(k p) h -> p k h", p=128))
                w1_sb[t] = w1t

            def load_w2(t):
                w2_t = []
                for g in range(2):
                    w = w2p.tile([128, 8 * D], BF16, tag="w2",
                                 name=f"w2_{t}{g}")
                    nc.sync.dma_start(
                        w[:].rearrange("p (k d) -> p k d", k=8),
                        w2[t][g * 1024:(g + 1) * 1024, :]
                        .rearrange("(k p) d -> p k d", p=128))
                    w2_t.append(w)
                w2_sb[t] = w2_t

            def load_b1(t):
                b1_sb[t] = cst.tile([128, MT1], F32R, tag=f"b1{t}",
                                    name=f"b1sb{t}")
                nc.sync.dma_start(b1_sb[t][:],
                                  b1[t].ap().rearrange("(m p) -> p m", p=128))

            def load_b2qk(t):
                b2qk_sb[t] = cst.tile([128, MT2], F32, tag=f"b2{t}",
                                      name=f"b2sb{t}")
                nc.sync.dma_start(b2qk_sb[t][:],
                                  b2[t].ap().rearrange("(m p) -> p m", p=128))

            xt = cst.tile([128, KT1 * ROWS], BF16, tag="xt")
            nc.sync.dma_start(xt[:].rearrange("p (k r) -> p k r", k=KT1),
                              xT.ap().rearrange("(k p) r -> p k r", p=128))
            load_w1("k")
            load_b1("k")
            load_b2qk("k")
            load_w2("k")
            load_w1("q")
            load_b1("q")
            load_b2qk("q")
            load_w2("q")
            load_b1("v")
            aux = cst.tile([1, 128 + D], BF16, tag="aux")
            ones128 = aux[:, 0:128]
            b2v_sb = aux[:, 128:128 + D]
            nc.sync.dma_start(aux[:, 0:128], ones_row_d[:])
            nc.sync.dma_start(
                b2v_sb, b2["v"].ap().rearrange("(a d) -> a d", a=1))
            tri_sb = cst.tile([128, 128], BF16, tag="tri")
            nc.sync.dma_start(tri_sb[:], tri_d[:])
            onescol = cst.tile([128, MT1], BF16, tag="onescol")
            nc.sync.dma_start(onescol[:], ones_col_d[:])
            load_w1("v")     # ring-slot wait (k-L1 done) is harmless here

            # ---- MLP helpers ----
            def mlp_l1(t, m_lo, m_hi, h1_t):
                for m in range(m_lo, m_hi):
                    pp = psp.tile([128, ROWS], F32, tag="ps",
                                  name=f"ps1_{t}{m}")
                    for kt in range(KT1):
                        nc.tensor.matmul(
                            pp[:],
                            w1_sb[t][:, kt * H + m * 128:
                                     kt * H + (m + 1) * 128],
                            xt[:, kt * ROWS:(kt + 1) * ROWS],
                            start=(kt == 0), stop=(kt == KT1 - 1))
                    h1 = h1p.tile([128, ROWS], BF16, tag="h1",
                                  name=f"h1_{t}{m}")
                    nc.scalar.activation(h1[:], pp[:], AF.Gelu_apprx_tanh,
                                         bias=b1_sb[t][:, m:m + 1])
                    h1_t.append(h1)

            def mlp_l2_T(t, h1_t):     # k, q: out = W2^T h1T + b2  [D, ROWS]
                w2_t = w2_sb[t]
                ot = l2p.tile([128, MT2 * ROWS], BF16, tag="l2",
                              name=f"l2_{t}")
                for m in range(MT2):
                    pp = psp.tile([128, ROWS], F32, tag="ps",
                                  name=f"ps2_{t}{m}")
                    for kt in range(MT1):
                        nc.tensor.matmul(
                            pp[:],
                            w2_t[kt // 8][:, (kt % 8) * D + m * 128:
                                          (kt % 8) * D + (m + 1) * 128],
                            h1_t[kt][:],
                            start=(kt == 0), stop=(kt == MT1 - 1))
                    with nc.allow_low_precision(reason="bf16 outputs"):
                        nc.vector.tensor_scalar_add(
                            ot[:, m * ROWS:(m + 1) * ROWS], pp[:],
                            b2qk_sb[t][:, m:m + 1])
                return ot

            def mlp_l2_v(h1_t):        # v: out = gelu(h1 W2 + b2)  [ROWS, D]
                w2_t = w2_sb["v"]
                ot = l2p.tile([128, MT2 * D], BF16, tag="l2", name="l2_v")
                for m in range(MT2):
                    pp = psp.tile([128, D], F32, tag="ps", name=f"ps2_v{m}")
                    for kt in range(MT1):
                        nc.tensor.matmul(
                            pp[:],
                            h1_t[kt][:, m * 128:(m + 1) * 128],
                            w2_t[kt // 8][:, (kt % 8) * D:(kt % 8 + 1) * D],
                            start=(kt == 0), stop=False)
                    nc.tensor.matmul(pp[:], ones128, b2v_sb,
                                     start=False, stop=True)
                    nc.scalar.activation(ot[:, m * D:(m + 1) * D], pp[:],
                                         AF.Gelu_apprx_tanh)
                    nc.scalar.dma_start(
                        send["v"][:, m * 128:(m + 1) * 128, :]
                        .rearrange("p r d -> r p d"),
                        ot[:, m * D:(m + 1) * D]
                        .rearrange("r (p d) -> r p d", p=N_CORES))
                return ot

            def send_T(t, ot):
                nc.scalar.dma_start(
                    send[t][:].rearrange("(m p) r -> p m r", p=128),
                    ot[:].rearrange("p (m r) -> p m r", m=MT2))
                if t == "k":
                    nc.scalar.dma_start(
                        kT_out.ap().rearrange("(m p) r -> p m r", p=128),
                        ot[:].rearrange("p (m r) -> p m r", m=MT2))

            def a2a(t):
                nc.gpsimd.collective_compute(
                    "AllToAll", mybir.AluOpType.bypass,
                    replica_groups=[list(range(N_CORES))],
                    ins=[send[t].opt()], outs=[recv[t].opt()])

            # kT/qT packed both batches: partitions [0:64] = batch 0,
            # [64:128] = batch 1 (enables row-tiled concurrent score MMs)
            kqT = {}

            def load_kqT(t, eng=None):
                # first 512 cols (k-rows 0-511, source cores 0-1) land in a
                # small first DMA so the first score tiles start ~4us sooner
                eng = eng or nc.sync
                tl = attp.tile([128, S], BF16, tag=f"{t}T", name=f"{t}T")
                for lo, hi in ((0, 2), (2, N_CORES)):
                    for b in range(N):
                        eng.dma_start(
                            tl[b * Dh:(b + 1) * Dh, lo * RPC:hi * RPC]
                            .rearrange("p (j r) -> p j r", j=hi - lo),
                            recv[t][lo * Dh:hi * Dh,
                                    b * RPC:(b + 1) * RPC]
                            .rearrange("(j p) r -> p j r", p=Dh))
                kqT[t] = tl

            # ---- k and q blocks ----
            h1_k, h1_q, h1_v = [], [], []
            mlp_l1("k", 0, MT1, h1_k)
            ot_k = mlp_l2_T("k", h1_k)
            load_w2("v")         # sync-ring-slot wait (k-L2 done)
            mlp_l1("q", 0, 2, h1_q)
            # ACT queue is just past the first q-L1 gelus here, and the k
            # outputs have been ready since k-L2 — triggers fire at once.
            # The a2a must be emitted after its send (deps follow emission).
            send_T("k", ot_k)
            a2a("k")
            load_kqT("k")        # sync ring; waits a2a(k), nothing behind
            mlp_l1("q", 2, MT1, h1_q)
            ot_q = mlp_l2_T("q", h1_q)
            vaug = {}
            for b in range(N):
                vaug[b] = vgp.tile([128, MT1 * 65], BF16, tag="vaug",
                                    name=f"va{b}")
                nc.sync.dma_start(
                    vaug[b][:].rearrange("p (g c) -> p g c", c=65)
                    [:, :, 64:65],
                    onescol[:].rearrange("p (g o) -> p g o", o=1))

            # scores+exp for one q-chunk: both batches packed per kt.
            # One persistent 4-bank psum tile; full-width kt tiles use its
            # halves alternately and share ONE [128,2048] exp per pair
            # (halves the per-instruction ACT overhead).
            PR = prp.tile([128, 2048], F32, tag="pr", name="pr")
            exps = {}        # (qc, kt) -> (sbuf tile, col offset)

            def scores(qc, kt_lo=0, kt_hi=None):
                q0 = qc * 512
                nk = 4 * qc + 4
                if kt_hi is None:
                    kt_hi = nk
                half = 0
                for kt in range(kt_lo, kt_hi):
                    o = max(0, kt * 128 - q0)   # causal column trim
                    w = 512 - o
                    pr = PR[:, half * 1024:(half + 1) * 1024]
                    for b in range(N):
                        nc.tensor.matmul(
                            pr[:, b * 512:b * 512 + w],
                            kqT["k"][b * Dh:(b + 1) * Dh,
                                     kt * 128:(kt + 1) * 128],
                            kqT["q"][b * Dh:(b + 1) * Dh,
                                     q0 + o:q0 + 512],
                            start=True, stop=True,
                            tile_position=(b * Dh, 0))
                    if w == 512 and half == 0 and kt + 1 < kt_hi \
                            and (kt + 1) * 128 < q0:
                        half = 1     # defer: pair with the next full kt
                        continue
                    if w == 512:
                        if half == 1:    # exp both halves in one instr
                            ex = expp2.tile([128, 2048], BF16, tag="exp2",
                                           name=f"ex2{qc}_{kt}")
                            nc.scalar.activation(ex[:], PR[:], AF.Exp,
                                                 scale=0.125)
                            exps[(qc, kt - 1)] = (ex, 0)
                            exps[(qc, kt)] = (ex, 1024)
                            half = 0
                            continue
                        ex = expp.tile([128, 1024], BF16, tag="exp",
                                       name=f"ex{qc}_{kt}")
                        nc.scalar.activation(ex[:], pr[:], AF.Exp,
                                             scale=0.125)
                    else:
                        ex = expp.tile([128, 1024], BF16, tag="exp",
                                       name=f"ex{qc}_{kt}")
                        rr = (lambda a: a.rearrange("p (b c) -> p b c", b=2)
                              [:, :, 0:w])
                        nc.scalar.activation(rr(ex[:]), rr(pr[:]), AF.Exp,
                                             scale=0.125)
                    half = 0
                    if kt * 128 >= q0:   # diagonal tile: causal mask
                        for b in range(N):
                            sl = ex[:, b * 512:b * 512 + 128]
                            nc.vector.tensor_mul(sl, sl, tri_sb[:])
                    exps[(qc, kt)] = (ex, 0)

            def pv(qc):
                q0 = qc * 512
                nk = 4 * qc + 4
                pob = {}
                for b in range(N):
                    pob[b] = pop.tile([65, 512], F32, tag="po",
                                      name=f"po{b}{qc}")
                for kt in range(nk):
                    o = max(0, kt * 128 - q0)
                    w = 512 - o
                    ex, off = exps[(qc, kt)]
                    for b in range(N):
                        nc.tensor.matmul(
                            pob[b][:, o:512],
                            vaug[b][:, kt * 65:(kt + 1) * 65],
                            ex[:, off + b * 512:off + b * 512 + w],
                            start=(kt == 0), stop=(kt == nk - 1))
                for b in range(N):
                    oT = smp.tile([65, 512], F32R, tag="oT", name=f"oT{b}{qc}")
                    nc.vector.tensor_copy(oT[:], pob[b][:])
                    nc.sync.dma_start(po_out[b, :, q0:q0 + 512], oT[:])

            # ---- v block; scores scheduled late so the PE stream covers
            # the a2a(v) + vaug-load latency before pv needs them ----
            mlp_l1("v", 0, 4, h1_v)
            send_T("q", ot_q)    # ACT reaches here just after q-L2 finishes
            a2a("q")
            load_kqT("q")
            mlp_l1("v", 4, MT1, h1_v)
            ot_v = mlp_l2_v(h1_v)
            a2a("v")
            # v_out is not needed by the collective — keep it off the
            # pre-collective DMA path entirely.
            nc.sync.dma_start(
                v_out.ap().rearrange("(m p) d -> p m d", p=128),
                ot_v[:].rearrange("p (m d) -> p m d", m=MT2))
            # qc3 first (its 16-tile exp chunk is the long ACT pole), then
            # in pv-consumption order so every exp lands before pv needs it
            scores(3)
            scores(0)
            scores(1)
            scores(2)
            for b in range(N):
                for h in range(2):
                    nc.gpsimd.dma_start(
                        vaug[b][:].rearrange("p (j h c) -> p j h c",
                                             j=N_CORES, h=2)[:, :, h, 0:64],
                        recv["v"][:, b * RPC + h * 128:
                                  b * RPC + (h + 1) * 128, :]
                        .rearrange("j p d -> p j d"))
            # consume in production order (3 first) so no pv waits on exps
            pv(3)
            pv(0)
            pv(1)
            pv(2)

    nc.compile()
    return nc


_COMPILED = None


def _get_compiled():
    global _COMPILED
    if _COMPILED is None:
        _COMPILED = _build()
    return _COMPILED


def _f32(a):
    return np.ascontiguousarray(np.asarray(a, dtype=np.float32))


def _bf16(a):
    return np.ascontiguousarray(np.asarray(a, dtype=np.float32)
                                .astype(ml_dtypes.bfloat16))


def _make_in_maps(x, qW1, qb1, qW2, qb2, kW1, kb1, kW2, kb2, vW1, vb1,
                  vW2, vb2):
    x = _f32(x)
    tri = (np.arange(128, dtype=np.int32)[None, :]
           >= np.arange(128, dtype=np.int32)[:, None]).astype(np.float32)
    shared = {
        "w1q": _bf16(qW1), "w1k": _bf16(kW1), "w1v": _bf16(vW1),
        "w2q": _bf16(qW2), "w2k": _bf16(kW2), "w2v": _bf16(vW2),
        "b1q": _f32(qb1), "b1k": _f32(kb1), "b1v": _f32(vb1),
        "b2q": _f32(qb2), "b2k": _f32(kb2), "b2v": _bf16(vb2),
        "tri": tri.astype(ml_dtypes.bfloat16),
        "ones_col": np.ones((128, MT1), ml_dtypes.bfloat16),
        "ones_row": np.ones((1, 128), ml_dtypes.bfloat16),
    }
    in_maps = []
    for c in range(N_CORES):
        xc = np.concatenate([x[b, c * RPC:(c + 1) * RPC, :]
                             for b in range(N)], 0)
        im = dict(shared)
        im["xT"] = np.ascontiguousarray(xc.T).astype(ml_dtypes.bfloat16)
        in_maps.append(im)
    return in_maps


def _assemble(res):
    k_full = np.empty((N, S, D), np.float32)
    v_full = np.empty((N, S, D), np.float32)
    out_full = np.empty((N, S, D), np.float32)
    for j in range(N_CORES):
        kT_j = np.asarray(res[j]["kT_out"], np.float32)   # [D, ROWS]
        v_j = np.asarray(res[j]["v_out"], np.float32)     # [ROWS, D]
        po_j = np.asarray(res[j]["attn_po"], np.float32)  # [N, 65, S]
        for b in range(N):
            k_full[b, j * RPC:(j + 1) * RPC, :] = \
                kT_j[:, b * RPC:(b + 1) * RPC].T
            v_full[b, j * RPC:(j + 1) * RPC, :] = v_j[b * RPC:(b + 1) * RPC, :]
            out_full[b, :, j * Dh:(j + 1) * Dh] = \
                (po_j[b, 0:Dh, :] / po_j[b, Dh:Dh + 1, :]).T
    return k_full, v_full, out_full


def kernel(**inputs):
    nc = _get_compiled()
    in_maps = _make_in_maps(**inputs)
    res = run_bass_kernel_spmd(nc, in_maps, list(range(N_CORES))).results
    return _assemble(res)
